# revision 56
# baseline (speedup 1.0000x reference)
"""Trainium2 Bass kernel for nn_Encoder_49658411876524 (Pyraformer-style
sparse-attention encoder).

Sharding (8 NeuronCores): core c computes batch c//2 with tensor-parallel
half c%2 (4 of 8 attention heads + half of the FFN hidden dim) over replica
groups [[0,1],[2,3],[4,5],[6,7]]. Per layer: one bf16 AllGather of the
attention outputs (issued per column chunk inside the attention loop, so it
hides behind remaining attention compute; fc then contracts all 8 heads
locally) and one bf16 AllReduce of the FFN partials (issued per column
chunk, pipelined against the next chunk's w2 matmuls).

Device algorithm (per core, transposed activations: D on SBUF partitions x
padded sequence 1408 on free axis; weights, state and attention tensors in
bf16 with fp32 PSUM accumulation; LN stats/residual adds in fp32):
- embedding conv/temporal/positional as one 32-contraction matmul
  (host-side im2col); bottleneck pyramid convs as strided matmuls;
  ELU = max(exp(min(x,0))-1, x)
- block-sparse attention: the pyramid mask decomposes into 29 dense
  sub-blocks (intra-level bands +-2 and parent/child links); per
  (chunk, head) the sub-block scores land packed in a double-buffered
  2-bank PSUM tile (512-wide parent blocks in their own bank) so exp and
  the 0/1 bf16 mask multiply run as ~3 batched ops instead of per-piece;
  softmax denominators and PV come from one accumulated matmul against
  V||ones; numerators are copied to oT unnormalized and scaled afterwards
- softmax denominators for all 4 heads land at partition rows 0/32/64/96
  of a ones-initialized tile (engine APs need 32-aligned partition bases);
  reciprocal via exp(-ln(d)) on the Act engine (ln/exp share one table);
  per-head broadcast via a (97,128) selector stationary matmul
- LayerNorm over the partition axis: per-chunk column sums via 1/D-scaled
  ones matmuls into rows 0/32/64 of persistent stat tiles, one batched
  var/rstd chain (rstd = exp(-0.5 ln(var+eps))), mean/rstd broadcast via
  selector matmuls, affine applied on the Act engine (Identity, per-
  partition scale/bias)
- per-layer weights double-buffered and prefetched during the previous
  layer's FFN; w2 streamed in two 4-chunk DMAs per layer (gpsimd dma
  dispatch costs ~0.7us each, so DMAs are batched)

NOTE: only the tp=True path is maintained (the harness entry point);
tp=False would need per-n w2 reloads (NKC=16 overflows the 2x4-chunk
double buffer).
"""
import sys
if '/opt/trn_rl_repo' not in sys.path:
    sys.path.insert(0, '/opt/trn_rl_repo')
import numpy as np
import concourse.bass as bass
import concourse.bacc as bacc
import concourse.mybir as mybir
import concourse.tile as tile


B, L_IN = 4, 1024
WINDOW = [4, 4, 4]
INNER = 5
ALL_SIZE = [1024, 256, 64, 16]
L_TOT = 1360
L_PAD = 1408
STARTS = [0, 1024, 1280, 1344]  # level starts


def get_mask():
    all_size = list(ALL_SIZE)
    L = sum(all_size)
    m = np.zeros((L, L), dtype=np.float32)
    iw = INNER // 2
    for li in range(len(all_size)):
        start = sum(all_size[:li])
        for i in range(start, start + all_size[li]):
            m[i, max(i - iw, start):min(i + iw + 1, start + all_size[li])] = 1
    for li in range(1, len(all_size)):
        start = sum(all_size[:li])
        for i in range(start, start + all_size[li]):
            ls = start - all_size[li - 1] + (i - start) * WINDOW[li - 1]
            if i == start + all_size[li] - 1:
                rs = start
            else:
                rs = start - all_size[li - 1] + (i - start + 1) * WINDOW[li - 1]
            m[i, ls:rs] = 1
            m[ls:rs, i] = 1
    return m.astype(bool)  # True = ALLOWED here


ALLOW = get_mask()


def gen_blocks():
    """Block pieces in scoresT orientation: (k0, kn, q0, qn).
    Each piece: keys [k0,k0+kn) on partitions, queries [q0,q0+qn) free.
    Returns list of (k0, kn, q0, qn). Query ranges are NOT yet chunk-split."""
    blocks = []
    iw = INNER // 2
    for li, (st, sz) in enumerate(zip(STARTS, ALL_SIZE)):
        # intra-level band: key tiles of <=128
        for kt0 in range(st, st + sz, 128):
            kn = min(128, st + sz - kt0)
            q0 = max(st, kt0 - iw)
            q1 = min(st + sz, kt0 + kn + iw)
            blocks.append((kt0, kn, q0, q1 - q0))
        # (q in this level, k in parent level): keys=parent level tiles,
        # queries = children span in this level
        if li + 1 < len(ALL_SIZE):
            pst, psz = STARTS[li + 1], ALL_SIZE[li + 1]
            w = WINDOW[li]
            for kt0 in range(pst, pst + psz, 128):
                kn = min(128, pst + psz - kt0)
                # parent key j (local i=j-pst) has children st+w*i .. st+w*i+w
                i0 = kt0 - pst
                q0 = st + w * i0
                q1 = min(st + sz, st + w * (i0 + kn))
                blocks.append((kt0, kn, q0, q1 - q0))
        # (q in this level, k in child level): keys=child level tiles,
        # queries = parents in this level
        if li >= 1:
            cst, csz = STARTS[li - 1], ALL_SIZE[li - 1]
            w = WINDOW[li - 1]
            for kt0 in range(cst, cst + csz, 128):
                kn = min(128, cst + csz - kt0)
                # child key j (local i=j-cst) has parent st + min(i//w, sz-1)
                i0 = kt0 - cst
                q0 = st + i0 // w
                q1 = st + min((i0 + kn - 1) // w, sz - 1) + 1
                blocks.append((kt0, kn, q0, q1 - q0))
    return blocks


def split_chunks(blocks, chunk=512):
    """Split query ranges at chunk boundaries."""
    out = []
    for (k0, kn, q0, qn) in blocks:
        q = q0
        while q < q0 + qn:
            qe = min(q0 + qn, (q // chunk + 1) * chunk)
            out.append((k0, kn, q, qe - q))
            q = qe
    return out


PACKW = 1536  # per-chunk packed score width (3 PSUM banks)


def pack_chunks(pieces, chunk=512, bank=512):
    """Pack each chunk's pieces into 2 512-col banks (no piece straddles a
    bank boundary); qn>256 pieces get the dedicated slot at off=1024 (their
    scores go to a separate 1-bank PSUM so the pack stays double-buffered).
    Returns per-piece (ci, off) with off in [0, PACKW), and per-chunk
    per-bank used widths (banks 0-1 only)."""
    used = [[0, 0, 0] for _ in range(3)]
    out = []
    for (k0, kn, q0, qn) in pieces:
        ci = q0 // chunk
        ubs = used[ci]
        if qn > 256:
            assert ubs[2] == 0, "one big piece per chunk"
            ubs[2] = qn
            out.append((ci, 2 * bank))
            continue
        for b in range(2):
            if ubs[b] + qn <= bank:
                off = b * bank + ubs[b]
                ubs[b] += qn
                break
        else:
            raise RuntimeError("pack overflow")
        out.append((ci, off))
    return out, used


def masks_packed(pieces, pack):
    """bf16 0/1 mask tile (128, 3*PACKW) at packed offsets."""
    import ml_dtypes
    m = np.zeros((128, 3 * PACKW), np.float32)
    for (k0, kn, q0, qn), (ci, off) in zip(pieces, pack):
        m[:kn, ci * PACKW + off:ci * PACKW + off + qn] = \
            ALLOW[q0:q0 + qn, k0:k0 + kn].T
    return m.astype(ml_dtypes.bfloat16)





F32 = mybir.dt.float32
F32R = mybir.dt.float32r
BF16 = mybir.dt.bfloat16
U8 = mybir.dt.uint8
AF = mybir.ActivationFunctionType
OP = mybir.AluOpType

D, H, DK, DFF, NL = 512, 8, 64, 2048, 4
L_IN, L_TOT, L_PAD = 1024, 1360, 1408
NDC = 4              # D chunks of 128
CHUNKS = [(0, 512), (512, 512), (1024, 336)]   # valid col ranges
NCH = [(0, 512), (512, 512), (1024, 384)]      # padded col ranges (to 1408)
INV_TEMP = 1.0 / 8.0
GROUPS = [[0, 1], [2, 3], [4, 5], [6, 7]]


def build(pieces, pack, used, tp=True, dbg=False, stop=None, n_layers=NL):
    """pieces: list of (k0,kn,q0,qn); pack: per-piece (ci, off) packed score
    column; used: per-chunk per-bank used widths."""
    HL = 4 if tp else 8          # local heads
    NHC = HL * DK // 128         # head chunks (2 or 4)
    NKC = 8 if tp else 16        # ffn w2 k chunks (DFF_local/128)
    nc = bacc.Bacc("TRN2", target_bir_lowering=False, debug=False, num_devices=8)

    def din(name, shape, dt=F32):
        return nc.dram_tensor(name, list(shape), dt, kind="ExternalInput")

    # ---- inputs ----
    embinT = din("embinT", (32, 1024))
    embw = din("embw", (32, D))
    addcT = din("addcT", (128, NDC, L_PAD))
    downw = din("downw", (128, NDC, 64))
    downb = din("downb", (64, 1))
    pyrw = din("pyrw", (64, 3, 4, 64))
    pyrb = din("pyrb", (64, 3))
    upw = din("upw", (64, D))
    upb = din("upb", (128, NDC))
    normg = din("normg", (128, NDC))
    normb = din("normb", (128, NDC))
    wq = din("wq", (NL, 128, NDC, HL * DK), BF16)
    wk = din("wk", (NL, 128, NDC, HL * DK), BF16)
    wv = din("wv", (NL, 128, NDC, HL * DK), BF16)
    fcw = din("fcw", (NL, 128, 4, NDC, 128), BF16)
    fcb = din("fcb", (NL, 128, NDC))
    ln1g = din("ln1g", (NL, 128, NDC))
    ln1b = din("ln1b", (NL, 128, NDC))
    w1 = din("w1", (NL, NKC, 128, NDC, 128), BF16)
    b1 = din("b1", (NL, 128, NKC))
    w2 = din("w2", (NL, NKC, 128, D), BF16)
    b2 = din("b2", (NL, 128, NDC))
    ln2g = din("ln2g", (NL, 128, NDC))
    ln2b = din("ln2b", (NL, 128, NDC))
    masks = din("masks", (128, 3 * PACKW), BF16)
    c01 = din("c01", (128, 2, 448))
    c01h = din("c01h", (128, 2, 448), BF16)
    sel97 = din("sel97", (97, 2, 128))
    seln = din("seln", (65, 3, 128))
    ones97 = din("ones97", (97, 512))
    cscal = din("cscal", (128, 2))
    oseq = nc.dram_tensor("oseq", [128, NDC, L_PAD], F32, kind="ExternalOutput")
    dbgs = {}

    def dout(name, shape):
        if name not in dbgs:
            dbgs[name] = nc.dram_tensor(name, list(shape), F32, kind="ExternalOutput")
        return dbgs[name]

    with tile.TileContext(nc) as tc, nc.allow_low_precision(
            reason="bf16 attention probabilities; rel-err gate is 2e-2"):
        _build_body(tc, locals(), pieces, pack, used,
                    tp, HL, NHC, NKC, dbg, dout, stop, n_layers)
    nc.compile()
    return nc


def _build_body(tc, T, pieces, pack, used, tp, HL, NHC,
                NKC, dbg, dout, stop=None, n_layers=NL):
    nc = tc.nc
    ctxs = []

    all_pools = []

    def pool(name, bufs, space="SBUF"):
        p = tc.alloc_tile_pool(name=name, bufs=bufs, space=space)
        all_pools.append(p)
        return p

    # ---- persistent pools ----
    pconst = pool("pconst", 1)          # params, masks, ones
    pstate = pool("pstate", 2)          # seqT ping/pong (f32r)
    parr = pool("parr", 1)              # pre-norm X / AR landing (f32)
    pqk = pool("pqk", 1)                # QT, KT, oT (f32r)
    pvau = pool("pvau", 1)              # Vaug
    pwl = pool("pwl", 1)                # per-layer weights (wq/wk/wv/fcw)
    ptrans = pool("ptrans", 2)          # (128,512) transients
    prow = pool("prow", 1)              # persistent stat rows
    prow2 = pool("prow2", 2)            # transient stat rows
    pbc = pool("pbc", 2)                # broadcast tiles (128,512)
    pdram = pool("pdram", 2, space="DRAM")
    pps = pool("pps", 2, space="PSUM")       # generic (128,512) + stat rows

    class Done(Exception):
        pass

    def finish(tile_):
        if tile_.dtype == BF16:
            tmp = parr.tile([128, NDC, L_PAD], F32R, tag="arr")
            for dc in range(NDC):
                nc.scalar.copy(tmp[:, dc, :], tile_[:, dc, :])
            nc.sync.dma_start(out=T["oseq"].ap(), in_=tmp[:].bitcast(F32))
        else:
            nc.sync.dma_start(out=T["oseq"].ap(), in_=tile_[:].bitcast(F32))
        for p in reversed(all_pools):
            p.release()

    # ---- consts (tiles alloc'd now; DMAs deferred past embedding loads) ----
    mtile = pconst.tile([128, 3 * PACKW], BF16)
    c_sel97 = pconst.tile([97, 2, 128], F32R)
    c_seln = pconst.tile([65, 3, 128], F32R)
    cscal_t = pconst.tile([128, 2], F32R)
    onesd = cscal_t[:, 0:1]
    epscol = cscal_t[:, 1:2]
    # softmax denominator tiles: rows 32*hh hold chunk ci denominators;
    # ones-init so in-place reciprocal keeps garbage rows finite (=1).
    rden_t = []
    for ci in range(3):
        t = pconst.tile([97, 512], F32R, name=f"rden{ci}", tag=f"rden{ci}")
        rden_t.append(t)

    def load_consts():
        nc.gpsimd.dma_start(out=mtile, in_=T["masks"].ap())
        nc.gpsimd.dma_start(out=c_sel97, in_=T["sel97"].ap().bitcast(F32R))
        nc.gpsimd.dma_start(out=c_seln, in_=T["seln"].ap().bitcast(F32R))
        nc.gpsimd.dma_start(out=cscal_t, in_=T["cscal"].ap().bitcast(F32R))
        for t in rden_t:
            nr = t.shape[0]
            nc.gpsimd.dma_start(out=t[:], in_=T["ones97"].ap()[0:nr].bitcast(F32R))

    def loadc(name, shape, dt=F32):
        t = pconst.tile(list(shape), dt, name=name, tag=name)
        src = T[name].ap()
        if dt == F32R:
            src = src.bitcast(F32R)
        nc.gpsimd.dma_start(out=t, in_=src)
        return t

    # param tiles
    c_normg = loadc("normg", (128, NDC))
    c_normb = loadc("normb", (128, NDC))

    st = pstate.tile([128, NDC, L_PAD], BF16, tag="state")

    # ================= embedding =================
    pemb = tc.alloc_tile_pool(name="pemb", bufs=1)
    e_w = pemb.tile([32, D], F32R)
    nc.gpsimd.dma_start(out=e_w, in_=T["embw"].ap().bitcast(F32R))
    e_downw = pemb.tile([128, NDC, 64], F32R)
    nc.gpsimd.dma_start(out=e_downw, in_=T["downw"].ap().bitcast(F32R))
    e_downb = pemb.tile([64, 1], F32)
    nc.gpsimd.dma_start(out=e_downb, in_=T["downb"].ap())
    e_pyrw = pemb.tile([64, 3, 4, 64], F32R)
    nc.gpsimd.dma_start(out=e_pyrw, in_=T["pyrw"].ap().bitcast(F32R))
    e_pyrb = pemb.tile([64, 3], F32)
    nc.gpsimd.dma_start(out=e_pyrb, in_=T["pyrb"].ap())
    e_upw = pemb.tile([64, D], F32R)
    nc.gpsimd.dma_start(out=e_upw, in_=T["upw"].ap().bitcast(F32R))
    e_upb = pemb.tile([128, NDC], F32)
    nc.gpsimd.dma_start(out=e_upb, in_=T["upb"].ap())
    load_consts()

    X0 = parr.tile([128, NDC, L_PAD], F32R, tag="arr")
    # X0 = embw.T @ embinT + addcT  (cols < 1024 only; rest zeroed)
    for n in range(2):
        n0 = n * 512
        einT = ptrans.tile([128, 512], F32R, tag="t2")
        nc.gpsimd.dma_start(out=einT[:32, :], in_=T["embinT"].ap()[:, n0:n0 + 512]
                            .bitcast(F32R))
        for mc in range(NDC):
            ps = pps.tile([128, 512], F32, tag="big")
            nc.tensor.matmul(ps[:, :512], e_w[:, mc * 128:(mc + 1) * 128],
                             einT[:32, :], start=True, stop=True)
            adc = ptrans.tile([128, 512], F32, tag="t1")
            nc.gpsimd.dma_start(out=adc[:, :512], in_=T["addcT"].ap()[:, mc, n0:n0 + 512])
            nc.vector.scalar_tensor_tensor(
                out=X0[:, mc, n0:n0 + 512], in0=ps[:, :512], scalar=0.0,
                in1=adc[:, :512], op0=OP.add, op1=OP.add)
    for mc in range(NDC):
        nc.gpsimd.dma_start(out=X0[:, mc, 1024:L_PAD], in_=T["c01"].ap()[:, 1, 0:L_PAD - 1024].bitcast(F32R))
    # dT = downw.T @ X0 + downb   (64 x L_PAD) -- only cols < 1024 used
    dT = pemb.tile([64, L_IN], F32R)
    for n in range(2):
        n0 = n * 512
        ps = pps.tile([128, 512], F32, tag="big")
        for dc in range(NDC):
            nc.tensor.matmul(ps[:64, :512], e_downw[:, dc, :],
                             X0[:, dc, n0:n0 + 512],
                             start=(dc == 0), stop=(dc == NDC - 1))
        nc.vector.tensor_scalar(out=dT[:, n0:n0 + 512], in0=ps[:64, :512],
                                scalar1=e_downb[:], scalar2=None, op0=OP.add)
    # pyramid levels
    pyrT = pemb.tile([64, 336], F32R)
    offs = [0, 256, 320]
    for lvl, Lo in enumerate([256, 64, 16]):
        cur = [dT[:, 0:1024], pyrT[:, 0:256], pyrT[:, 256:320]][lvl]
        ps = pps.tile([128, 512], F32, tag="big")
        curv = cur.rearrange("p (t k) -> p k t", k=4)
        for k in range(4):
            nc.tensor.matmul(ps[:64, :Lo], e_pyrw[:, lvl, k, :], curv[:, k, :],
                             start=(k == 0), stop=(k == 3))
        # ELU: out = max(exp(min(x+b,0)) - 1, x+b); add bias first
        xb = ptrans.tile([128, 512], F32, tag="t1")
        nc.vector.tensor_scalar(out=xb[:64, :Lo], in0=ps[:64, :Lo],
                                scalar1=e_pyrb[:, lvl:lvl + 1], scalar2=None,
                                op0=OP.add)
        mn = ptrans.tile([128, 512], F32, tag="t2", bufs=2)
        nc.vector.tensor_scalar(out=mn[:64, :Lo], in0=xb[:64, :Lo],
                                scalar1=0.0, scalar2=None, op0=OP.min)
        ex = ptrans.tile([128, 512], F32, tag="t2", bufs=2)
        nc.scalar.activation(ex[:64, :Lo], mn[:64, :Lo], AF.Exp)
        o0 = offs[lvl]
        nc.vector.scalar_tensor_tensor(
            out=pyrT[:, o0:o0 + Lo], in0=ex[:64, :Lo], scalar=1.0,
            in1=xb[:64, :Lo], op0=OP.subtract, op1=OP.max)
    # up: X0[:, :, 1024:1360] = upw.T @ pyrT + upb
    for mc in range(NDC):
        ps = pps.tile([128, 512], F32, tag="big")
        nc.tensor.matmul(ps[:, :336], e_upw[:, mc * 128:(mc + 1) * 128],
                         pyrT[:], start=True, stop=True)
        nc.vector.tensor_scalar(out=X0[:, mc, 1024:1360], in0=ps[:, :336],
                                scalar1=e_upb[:, mc:mc + 1], scalar2=None,
                                op0=OP.add)
    if dbg:
        nc.gpsimd.dma_start(out=dout("dbg_X0", (128, NDC, L_PAD)).ap(), in_=X0[:].bitcast(F32))

    # ---- LayerNorm helper (over D=partition axis via ones-matmul) ----
    _ln_id = [0]

    def layer_norm(X, g_ap, b_ap, out_tile):
        """X: (128,NDC,L_PAD) f32 tile; g/b: (128,NDC) APs; out f32r tile.
        Stats packed at partition rows 32*chunk; one batched reciprocal;
        partition-broadcast via selector matmuls; affine on Act engine."""
        _ln_id[0] += 1
        pln = tc.alloc_tile_pool(name=f"pln{_ln_id[0]}", bufs=2, space="PSUM")
        # fully per-chunk: chunk n's normalize (and its consumers) never
        # wait on later chunks' stats -- lets QKV/FFN pipeline across the
        # collectives that feed X chunk by chunk
        onesr = c_seln[0:1, 0, :]
        for n, (n0, nw) in enumerate(NCH):
            sps = pps.tile([1, 512], F32, tag="big")
            qps = pps.tile([1, 512], F32, tag="big")
            for dc in range(NDC):
                xsq = ptrans.tile([128, 512], F32R, tag="t1")
                nc.vector.tensor_mul(xsq[:, :nw], X[:, dc, n0:n0 + nw],
                                     X[:, dc, n0:n0 + nw])
                nc.tensor.matmul(sps[:, :nw], onesd,
                                 X[:, dc, n0:n0 + nw],
                                 start=(dc == 0), stop=(dc == NDC - 1))
                nc.tensor.matmul(qps[:, :nw], onesd, xsq[:, :nw],
                                 start=(dc == 0), stop=(dc == NDC - 1))
            mu = prow2.tile([1, 512], F32R, tag="mu", bufs=2)
            nc.scalar.copy(mu[:, :nw], sps[:, :nw])
            mu2 = prow2.tile([1, 512], F32R, tag="mu2", bufs=2)
            nc.vector.tensor_mul(mu2[:, :nw], mu[:, :nw], mu[:, :nw])
            var = prow2.tile([1, 512], F32R, tag="var", bufs=2)
            nc.vector.scalar_tensor_tensor(
                out=var[:, :nw], in0=qps[:, :nw], scalar=0.0,
                in1=mu2[:, :nw], op0=OP.add, op1=OP.subtract)
            # rstd = (var+eps)^-0.5 via exp(-0.5*ln(var+eps)) on Act
            nc.scalar.activation(var[:, :nw], var[:, :nw], AF.Ln,
                                 bias=epscol[0:1])
            rstd = prow2.tile([1, 512], F32R, tag="rstd", bufs=2)
            nc.scalar.activation(rstd[:, :nw], var[:, :nw], AF.Exp, scale=-0.5)
            mub = pln.tile([128, 512], F32, tag="mub")
            nc.tensor.matmul(mub[:, :nw], onesr, mu[:, :nw],
                             start=True, stop=True)
            rsb = pln.tile([128, 512], F32, tag="rsb")
            nc.tensor.matmul(rsb[:, :nw], onesr, rstd[:, :nw],
                             start=True, stop=True)
            for dc in range(NDC):
                t = ptrans.tile([128, 512], F32, tag="t1")
                nc.vector.tensor_sub(t[:, :nw], X[:, dc, n0:n0 + nw], mub[:, :nw])
                t2 = ptrans.tile([128, 512], F32, tag="t2", bufs=2)
                nc.vector.tensor_mul(t2[:, :nw], t[:, :nw], rsb[:, :nw])
                nc.scalar.activation(out_tile[:, dc, n0:n0 + nw], t2[:, :nw],
                                     AF.Identity, bias=b_ap[:, dc:dc + 1],
                                     scale=g_ap[:, dc:dc + 1])
        pln.release()

    pemb.release()
    if stop == "emb":
        finish(X0)
        return
    layer_norm(X0, c_normg[:], c_normb[:], st)
    if stop == "ln0":
        finish(st)
        return
    if dbg:
        nc.gpsimd.dma_start(out=dout("dbg_seq0", (128, NDC, L_PAD)).ap(),
                          in_=st[:].bitcast(F32))

    # ================= encoder layers =================
    # hoisted tiles: reused across layers (WAW-dep gated, no slot stalls)
    def load_layer(l):
        t_wq = pwl.tile([128, NDC, HL * DK], BF16, tag="wq", bufs=2, name="t_wq")
        t_wk = pwl.tile([128, NDC, HL * DK], BF16, tag="wk", bufs=2, name="t_wk")
        t_wv = pwl.tile([128, NDC, HL * DK], BF16, tag="wv", bufs=2, name="t_wv")
        t_fcw = pwl.tile([128, 4, NDC, 128], BF16, tag="fcw", bufs=2, name="t_fcw")
        t_w1f = pwl.tile([128, NKC, NDC, 128], BF16, tag="w1f", bufs=2, name="t_w1f")
        nc.gpsimd.dma_start(out=t_wq, in_=T["wq"].ap()[l])
        nc.gpsimd.dma_start(out=t_wk, in_=T["wk"].ap()[l])
        nc.gpsimd.dma_start(out=t_wv, in_=T["wv"].ap()[l])
        nc.gpsimd.dma_start(out=t_fcw, in_=T["fcw"].ap()[l])
        nc.gpsimd.dma_start(out=t_w1f, in_=T["w1"].ap()[l].rearrange("k p c m -> p k c m"))
        return (t_wq, t_wk, t_wv, t_fcw, t_w1f)
    c_fcb = pwl.tile([128, NL, NDC], F32, tag="fcb")
    nc.gpsimd.dma_start(out=c_fcb, in_=T["fcb"].ap().rearrange("l p c -> p l c"))
    c_ln1g = pwl.tile([128, NL, NDC], F32, tag="ln1g")
    nc.gpsimd.dma_start(out=c_ln1g, in_=T["ln1g"].ap().rearrange("l p c -> p l c"))
    c_ln1b = pwl.tile([128, NL, NDC], F32, tag="ln1b")
    nc.gpsimd.dma_start(out=c_ln1b, in_=T["ln1b"].ap().rearrange("l p c -> p l c"))
    c_b1 = pwl.tile([128, NL, NKC], F32, tag="b1")
    nc.gpsimd.dma_start(out=c_b1, in_=T["b1"].ap().rearrange("l p c -> p l c"))
    c_b2 = pwl.tile([128, NL, NDC], F32, tag="b2")
    nc.gpsimd.dma_start(out=c_b2, in_=T["b2"].ap().rearrange("l p c -> p l c"))
    c_ln2g = pwl.tile([128, NL, NDC], F32, tag="ln2g")
    nc.gpsimd.dma_start(out=c_ln2g, in_=T["ln2g"].ap().rearrange("l p c -> p l c"))
    c_ln2b = pwl.tile([128, NL, NDC], F32, tag="ln2b")
    nc.gpsimd.dma_start(out=c_ln2b, in_=T["ln2b"].ap().rearrange("l p c -> p l c"))
    t_w2a = pwl.tile([128, 4, D], BF16, tag="w2a")
    t_w2b = pwl.tile([128, 4, D], BF16, tag="w2b")
    QT = pqk.tile([128, NHC, L_PAD], BF16, tag="QT")
    KT = pqk.tile([128, NHC, L_PAD], BF16, tag="KT")
    oT = pqk.tile([128, NHC, L_PAD], BF16, tag="oT")
    oTf = pqk.tile([128, 4, L_PAD], BF16, tag="oTf")
    nc.gpsimd.dma_start(out=oTf[:, :, L_TOT:L_PAD], in_=T["c01h"].ap()[:, 1, 0:4 * (L_PAD - L_TOT)].rearrange("p (a b) -> p a b", a=4))
    nc.gpsimd.dma_start(out=oT[:, :, L_TOT:L_PAD], in_=T["c01h"].ap()[:, 1, 0:NHC * (L_PAD - L_TOT)].rearrange("p (a b) -> p a b", a=NHC))
    Vau = pvau.tile([128, 11, HL, DK + 1], BF16, tag="V")
    nc.gpsimd.dma_start(out=Vau[:, :, :, DK:DK + 1], in_=T["c01h"].ap()[:, 0, 0:11 * HL].rearrange("p (a b) -> p a b", a=11).unsqueeze(3))
    vau3 = pvau.tile([128, HL, DK + 1], BF16, tag="V3")
    nc.gpsimd.dma_start(out=vau3[:, :, DK:DK + 1], in_=T["c01h"].ap()[:, 0, 0:HL].unsqueeze(2))

    tiles_l = load_layer(0)
    for l in range(n_layers):
        t_wq, t_wk, t_wv, t_fcw, t_w1f = tiles_l
        t_fcb = c_fcb[:, l]
        t_ln1g = c_ln1g[:, l]
        t_ln1b = c_ln1b[:, l]
        t_b1 = c_b1[:, l]
        t_b2 = c_b2[:, l]
        t_ln2g = c_ln2g[:, l]
        t_ln2b = c_ln2b[:, l]

        # --- QKV projections (chunk loop outermost: chunk-0/1 work issues
        # while the previous LN2 is still finishing chunk 2) ---
        for n, (n0, nw) in enumerate(NCH):
            for dst, w in ((QT, t_wq), (KT, t_wk)):
                for hc in range(NHC):
                    ps = pps.tile([128, 512], F32, tag="big")
                    for dc in range(NDC):
                        nc.tensor.matmul(ps[:, :nw],
                                         w[:, dc, hc * 128:(hc + 1) * 128],
                                         st[:, dc, n0:n0 + nw],
                                         start=(dc == 0), stop=(dc == NDC - 1))
                    nc.vector.tensor_scalar(out=dst[:, hc, n0:n0 + nw],
                                            in0=ps[:, :nw], scalar1=0.0,
                                            scalar2=None, op0=OP.add)
        for lt in range(11):
            ps = pps.tile([128, 512], F32, tag="big")
            for dc in range(NDC):
                nc.tensor.matmul(ps[:, :HL * DK],
                                 st[:, dc, lt * 128:(lt + 1) * 128],
                                 t_wv[:, dc, :], start=(dc == 0),
                                 stop=(dc == NDC - 1))
            nc.vector.tensor_scalar(out=Vau[:, lt, :, 0:DK],
                                    in0=ps[:, :HL * DK]
                                    .rearrange("p (h d) -> p h d", h=HL),
                                    scalar1=0.0, scalar2=None, op0=OP.add)
        nc.gpsimd.dma_start(out=vau3[0:16, :, 0:DK], in_=Vau[64:80, 10, :, 0:DK])

        if stop == "qkv" and l == 0:
            finish(st)
            return
        # --- attention: packed-PSUM scores, batched exp/mask, bf16 ---
        ppso = tc.alloc_tile_pool(name="ppso", bufs=1, space="PSUM")
        ppsa = tc.alloc_tile_pool(name="ppsa", bufs=2, space="PSUM")
        pexp = tc.alloc_tile_pool(name="pexp", bufs=2)
        chunk_grps = [[i for i, (ci, off) in enumerate(pack) if ci == c]
                      for c in range(3)]
        for ci, (c0, cw) in enumerate(CHUNKS):
            grp = chunk_grps[ci]
            ubs = used[ci]
            rden = rden_t[ci]
            for hh in range(HL):
                hc, hp = hh // 2, (hh % 2) * 64
                sp = ppso.tile([128, 1024], F32, tag="sp", bufs=2)
                bigp = None
                for i in grp:
                    k0, kn, q0, qn = pieces[i]
                    off = pack[i][1]
                    if off == 1024:
                        bigp = pps.tile([128, 512], F32, tag="big")
                        nc.tensor.matmul(
                            bigp[:kn, :qn], KT[hp:hp + 64, hc, k0:k0 + kn],
                            QT[hp:hp + 64, hc, q0:q0 + qn], start=True, stop=True)
                    else:
                        nc.tensor.matmul(
                            sp[:kn, off:off + qn], KT[hp:hp + 64, hc, k0:k0 + kn],
                            QT[hp:hp + 64, hc, q0:q0 + qn], start=True, stop=True)
                mep = pexp.tile([128, PACKW], BF16, tag="mep")
                for b in range(3):
                    if ubs[b] == 0:
                        continue
                    b0 = b * 512
                    src = bigp[:, :ubs[b]] if b == 2 else sp[:, b0:b0 + ubs[b]]
                    ex = pexp.tile([128, 512], BF16, tag="ex", bufs=2)
                    nc.scalar.activation(ex[:, :ubs[b]], src,
                                         AF.Exp, scale=INV_TEMP)
                    nc.vector.tensor_mul(mep[:, b0:b0 + ubs[b]], ex[:, :ubs[b]],
                                         mtile[:, ci * PACKW + b0:ci * PACKW + b0 + ubs[b]])
                acc = ppsa.tile([DK + 1, 512], F32, tag="acc")
                for gi, i in enumerate(grp):
                    k0, kn, q0, qn = pieces[i]
                    off = pack[i][1]
                    lt, ko = k0 // 128, k0 % 128
                    vlhs = (Vau[ko:ko + kn, lt, hh, :] if ko == 0
                            else vau3[0:kn, hh, :])
                    nc.tensor.matmul(acc[:, q0 - c0:q0 - c0 + qn],
                                     vlhs,
                                     mep[:kn, off:off + qn], start=(gi == 0),
                                     stop=(gi == len(grp) - 1))
                # unnormalized numerators -> oT; denominator -> rden row 32*hh
                nc.vector.tensor_scalar(out=oT[hp:hp + 64, hc, c0:c0 + cw],
                                        in0=acc[0:DK, :cw], scalar1=0.0,
                                        scalar2=None, op0=OP.add)
                nc.scalar.copy(rden[32 * hh:32 * hh + 1, :cw],
                               acc[DK:DK + 1, :cw])
            # reciprocal of all 4 heads (rows 0/32/64/96) via exp(-ln(d)) on
            # the Act engine (ln+exp share a table set; DVE recip is ~3.4us)
            nc.scalar.activation(rden[:], rden[:], AF.Ln)
            nc.scalar.activation(rden[:], rden[:], AF.Exp, scale=-1.0)
            for hc2 in range(NHC):
                rb = pps.tile([128, 512], F32, tag="big")
                nc.tensor.matmul(rb[:, :cw], c_sel97[:, hc2, :], rden[:, :cw],
                                 start=True, stop=True)
                nc.vector.tensor_mul(oT[:, hc2, c0:c0 + cw],
                                     oT[:, hc2, c0:c0 + cw], rb[:, :cw])
            # kick the per-chunk AllGather of oT as soon as this chunk is
            # normalized -- overlaps remaining attention chunks and fc
            if tp:
                cing = pdram.tile([128, NHC, cw], BF16, tag=f"cing{ci}")
                coutg = pdram.tile([2, 128, NHC, cw], BF16, tag=f"coutg{ci}")
                nc.sync.dma_start(out=cing[:], in_=oT[:, :, c0:c0 + cw])
                nc.gpsimd.collective_compute(
                    "AllGather", OP.bypass, replica_groups=GROUPS,
                    ins=[cing.opt()], outs=[coutg.opt()])
                for r in range(2):
                    nc.sync.dma_start(out=oTf[:, 2 * r:2 * r + 2, c0:c0 + cw],
                                      in_=coutg[r])
        pexp.release()
        ppsa.release()
        ppso.release()
        if stop == "attn" and l == 0:
            finish(st)
            return
        # --- full-contraction fc (per-chunk, pipelined behind the AG) ---
        if tp:
            # pad cols beyond L_TOT in oTf are never gathered; zero once
            fc_src = oTf
        else:
            fc_src = oT
        arr = parr.tile([128, NDC, L_PAD], F32R, tag="arr")
        for n, (n0, nw) in enumerate(NCH):
            for mc in range(NDC):
                ps = pps.tile([128, 512], F32, tag="big")
                for kc in range(4):
                    nc.tensor.matmul(ps[:, :nw], t_fcw[:, kc, mc, :],
                                     fc_src[:, kc, n0:n0 + nw],
                                     start=(kc == 0), stop=(kc == 3))
                nc.scalar.copy(arr[:, mc, n0:n0 + nw], ps[:, :nw])
        if dbg and l == 0:
            nc.gpsimd.dma_start(out=dout("dbg_fcp", (128, NDC, L_PAD)).ap(),
                              in_=arr[:].bitcast(F32))
        if stop == "fc" and l == 0:
            finish(arr)
            return
        if stop == "ar1" and l == 0:
            finish(arr)
            return
        res = st
        st2 = pstate.tile([128, NDC, L_PAD], BF16, tag="state")
        for dc in range(NDC):
            nc.vector.scalar_tensor_tensor(
                out=arr[:, dc, :], in0=arr[:, dc, :],
                scalar=t_fcb[:, dc:dc + 1], in1=res[:, dc, :],
                op0=OP.add, op1=OP.add)
        layer_norm(arr, t_ln1g, t_ln1b, st2)
        if stop == "ln1" and l == 0:
            finish(st2)
            return
        if dbg and l == 0:
            nc.gpsimd.dma_start(out=dout("dbg_ln1", (128, NDC, L_PAD)).ap(),
                              in_=st2[:].bitcast(F32))

        # --- FFN: w1 -> gelu -> w2 partial -> AR(bf16) -> +b2 +res -> LN2 ---
        arr2h = parr.tile([128, NDC, L_PAD], BF16, tag="arrh")
        ppsf = tc.alloc_tile_pool(name="ppsf", bufs=4, space="PSUM")
        for n, (n0, nw) in enumerate(NCH):
            fps = [ppsf.tile([128, 512], F32, tag="fc", name=f"fps{_i}") for _i in range(NDC)]
            for kc in range(NKC):
                t_w2 = t_w2a if (kc // 4) % 2 == 0 else t_w2b
                if kc % 4 == 0 and n == 0:
                    nc.gpsimd.dma_start(
                        out=t_w2,
                        in_=T["w2"].ap()[l, kc:kc + 4].rearrange("k p m -> p k m"))
                hps = pps.tile([128, 512], F32, tag="big")
                for dc in range(NDC):
                    nc.tensor.matmul(hps[:, :nw], t_w1f[:, kc, dc, :],
                                     st2[:, dc, n0:n0 + nw],
                                     start=(dc == 0), stop=(dc == NDC - 1))
                hm = ptrans.tile([128, 512], BF16, tag="t1")
                nc.scalar.activation(hm[:, :nw], hps[:, :nw], AF.Gelu,
                                     bias=t_b1[:, kc:kc + 1])
                for mc in range(NDC):
                    nc.tensor.matmul(fps[mc][:, :nw],
                                     t_w2[:, kc % 4, mc * 128:(mc + 1) * 128],
                                     hm[:, :nw], start=(kc == 0),
                                     stop=(kc == NKC - 1))
            for mc in range(NDC):
                nc.scalar.copy(arr2h[:, mc, n0:n0 + nw], fps[mc][:, :nw])
            if tp:
                # per-chunk AllReduce, pipelined with the next chunk's w2
                cin2 = pdram.tile([128, NDC, nw], BF16, tag=f"cin{n}")
                cout2 = pdram.tile([128, NDC, nw], BF16, tag=f"cout{n}")
                nc.sync.dma_start(out=cin2[:], in_=arr2h[:, :, n0:n0 + nw])
                nc.gpsimd.collective_compute(
                    "AllReduce", OP.add, replica_groups=GROUPS,
                    ins=[cin2.opt()], outs=[cout2.opt()])
                nc.sync.dma_start(out=arr2h[:, :, n0:n0 + nw], in_=cout2[:])
        ppsf.release()
        if l + 1 < n_layers:
            tiles_l = load_layer(l + 1)
        res2 = st2
        arr2 = parr.tile([128, NDC, L_PAD], F32R, tag="arr")
        st3 = pstate.tile([128, NDC, L_PAD], BF16, tag="state")
        for n, (n0, nw) in enumerate(NCH):
            for dc in range(NDC):
                nc.vector.scalar_tensor_tensor(
                    out=arr2[:, dc, n0:n0 + nw], in0=arr2h[:, dc, n0:n0 + nw],
                    scalar=t_b2[:, dc:dc + 1], in1=res2[:, dc, n0:n0 + nw],
                    op0=OP.add, op1=OP.add)
        layer_norm(arr2, t_ln2g, t_ln2b, st3)
        st = st3

    finish(st)



B, L_IN, C_IN, C_MARK = 4, 1024, 7, 4
D, H, DK, DFF, NL = 512, 8, 64, 2048, 4
L_TOT, L_PAD, NDC = 1360, 1408, 4
WINDOW = [4, 4, 4]

_CACHE = {}


def get_pieces():
    if 'pieces' not in _CACHE:
        pieces = split_chunks(gen_blocks())
        pack, used = pack_chunks(pieces)
        packed = masks_packed(pieces, pack)
        _CACHE['pieces'] = (pieces, pack, used, packed)
    return _CACHE['pieces']


def _pe():
    pos = np.arange(L_IN, dtype=np.float32)[:, None]
    div = np.exp(np.arange(0, D, 2, dtype=np.float32) * (-np.log(10000.0) / D))
    pe = np.zeros((L_IN, D), np.float32)
    pe[:, 0::2] = np.sin(pos * div)
    pe[:, 1::2] = np.cos(pos * div)
    return pe


def chunkT(M):
    """(512, L) -> (128, NDC, L) device layout."""
    Dd, L = M.shape
    return np.ascontiguousarray(M.reshape(NDC, 128, L).transpose(1, 0, 2))


def chunk_vec(v):
    """(512,) -> (128, NDC)."""
    return np.ascontiguousarray(np.asarray(v, np.float32).reshape(NDC, 128).T)


def _sel97():
    s = np.zeros((97, 2, 128), np.float32)
    for hc in range(2):
        s[32 * (2 * hc), hc, 0:64] = 1.0
        s[32 * (2 * hc + 1), hc, 64:128] = 1.0
    return s


def _seln():
    s = np.zeros((65, 3, 128), np.float32)
    for n in range(3):
        s[32 * n, n, :] = 1.0
    return s


def host_prep(inputs, b, h, tp=True):
    import ml_dtypes
    f = lambda x: np.asarray(x, np.float32)
    HL = 4 if tp else 8
    NHC = HL * DK // 128
    NKC = (DFF // 2 if tp else DFF) // 128
    DFL = NKC * 128
    hs = h * HL * DK if tp else 0
    fs = h * DFL if tp else 0
    x_enc, x_mark = f(inputs['x_enc'][b]), f(inputs['x_mark'][b])
    conv_w, conv_b = f(inputs['conv_w']), f(inputs['conv_b'])
    temp_w, temp_b = f(inputs['temp_w']), f(inputs['temp_b'])
    embinT = np.zeros((32, 1024), np.float32)
    for k in range(3):
        embinT[k * 7:(k + 1) * 7, :L_IN] = np.roll(x_enc, 1 - k, axis=0).T
    embinT[21:25, :L_IN] = x_mark.T
    embw = np.zeros((32, D), np.float32)
    for k in range(3):
        embw[k * 7:(k + 1) * 7, :] = conv_w[:, :, k].T
    embw[21:25, :] = temp_w
    addcT = np.zeros((D, L_PAD), np.float32)
    addcT[:, :L_IN] = _pe().T + (conv_b + temp_b)[:, None]
    bn_g, bn_b = f(inputs['bn_g']), f(inputs['bn_b'])
    pcw, pcb = f(inputs['pyr_conv_w']), f(inputs['pyr_conv_b'])
    pyrw = np.zeros((64, 3, 4, 64), np.float32)
    pyrb = np.zeros((64, 3), np.float32)
    for lvl in range(3):
        for k in range(4):
            pyrw[:, lvl, k, :] = pcw[lvl][:, :, k].T * bn_g[lvl][None, :]
        pyrb[:, lvl] = pcb[lvl] * bn_g[lvl] + bn_b[lvl]
    wq = f(inputs['wq'])[:, :, hs:hs + HL * DK]
    wk = f(inputs['wk'])[:, :, hs:hs + HL * DK]
    wv = f(inputs['wv'])[:, :, hs:hs + HL * DK]
    fcw_s = f(inputs['fc_w'])
    w1_s = f(inputs['w1'])[:, :, fs:fs + DFL]
    b1_s = f(inputs['b1'])[:, fs:fs + DFL]
    w2_s = f(inputs['w2'])[:, fs:fs + DFL, :]
    d = dict(
        embinT=embinT, embw=embw, addcT=chunkT(addcT),
        downw=chunkT(f(inputs['down_w'])), downb=f(inputs['down_b'])[:, None],
        pyrw=pyrw, pyrb=pyrb, upw=f(inputs['up_w']),
        upb=chunk_vec(inputs['up_b']),
        normg=chunk_vec(inputs['norm_g']), normb=chunk_vec(inputs['norm_b']),
        wq=np.stack([chunkT(wq[l]) for l in range(NL)]).astype(ml_dtypes.bfloat16),
        wk=np.stack([chunkT(wk[l]) for l in range(NL)]).astype(ml_dtypes.bfloat16),
        wv=np.stack([chunkT(wv[l]) for l in range(NL)]).astype(ml_dtypes.bfloat16),
        fcw=np.ascontiguousarray(
            fcw_s.reshape(NL, 4, 128, NDC, 128).transpose(0, 2, 1, 3, 4)
        ).astype(ml_dtypes.bfloat16),
        fcb=np.stack([chunk_vec(inputs['fc_b'][l]) for l in range(NL)]),
        ln1g=np.stack([chunk_vec(inputs['ln1_g'][l]) for l in range(NL)]),
        ln1b=np.stack([chunk_vec(inputs['ln1_b'][l]) for l in range(NL)]),
        w1=np.ascontiguousarray(
            w1_s.reshape(NL, NDC, 128, NKC, 128).transpose(0, 3, 2, 1, 4)).astype(ml_dtypes.bfloat16),
        b1=np.ascontiguousarray(b1_s.reshape(NL, NKC, 128).transpose(0, 2, 1)),
        w2=np.ascontiguousarray(w2_s.reshape(NL, NKC, 128, D)).astype(ml_dtypes.bfloat16),
        b2=np.stack([chunk_vec(inputs['b2'][l]) for l in range(NL)]),
        ln2g=np.stack([chunk_vec(inputs['ln2_g'][l]) for l in range(NL)]),
        ln2b=np.stack([chunk_vec(inputs['ln2_b'][l]) for l in range(NL)]),
        masks=get_pieces()[3],
        c01=np.stack([np.ones((128, 448), np.float32), np.zeros((128, 448), np.float32)], 1),
        c01h=np.stack([np.ones((128, 448), ml_dtypes.bfloat16),
                       np.zeros((128, 448), ml_dtypes.bfloat16)], 1),
        sel97=_sel97(),
        seln=_seln(),
        ones97=np.ones((97, 512), np.float32),
        cscal=np.stack([np.full(128, 1.0 / D, np.float32), np.full(128, 1e-5, np.float32)], 1),
    )
    _bf = ('masks', 'c01h', 'fcw', 'wq', 'wk', 'wv', 'w1', 'w2')
    return {k: np.ascontiguousarray(v, dtype=(ml_dtypes.bfloat16 if k in _bf else np.float32))
            for k, v in d.items()}


def refer_idx():
    all_sizes = [1024, 256, 64, 16]
    idx = np.zeros((1024, 4), dtype=np.int64)
    for i in range(1024):
        idx[i, 0] = i
        former = i
        for j in range(1, 4):
            start = sum(all_sizes[:j])
            inner = former - (start - all_sizes[j - 1])
            former = start + min(inner // WINDOW[j - 1], all_sizes[j] - 1)
            idx[i, j] = former
    return idx


def get_nc(tp=True, dbg=False, stop=None, n_layers=4):
    key = ('nc', tp, dbg, stop, n_layers)
    if key not in _CACHE:
        pieces, pack, used, packed = get_pieces()
        _CACHE[key] = build(pieces, pack, used, tp=tp, dbg=dbg, stop=stop, n_layers=n_layers)
    return _CACHE[key]


def _run(trace=False, tmpdir=None, tp=True, dbg=False, stop=None, n_layers=4, raw=False, **inputs):
    from concourse.bass_utils import run_bass_kernel_spmd
    nc = get_nc(tp=tp, dbg=dbg, stop=stop, n_layers=n_layers)
    in_maps = []
    for c in range(8):
        b, h = c // 2, c % 2
        in_maps.append(host_prep(inputs, b, h, tp=tp))
    res = run_bass_kernel_spmd(nc, in_maps, list(range(8)), trace=trace,
                               tmpdir=tmpdir)
    if raw:
        return res
    outs = []
    for b in range(B):
        o = res.results[2 * b]["oseq"]            # (128, NDC, L_PAD)
        M = o.transpose(1, 0, 2).reshape(D, L_PAD)
        outs.append(M[:, :L_TOT].T)               # (1360, 512)
    seq = np.stack(outs)
    idx = refer_idx()
    out = seq[:, idx, :].reshape(B, L_IN, 4 * D).astype(np.float32)
    return out, res


def axon_reset():
    import ctypes, jax
    jax.devices()
    lib = ctypes.CDLL('/opt/axon/libaxon_pjrt.so')
    lib.axon_reset.restype = ctypes.c_int64
    return lib.axon_reset()


def kernel(**inputs):
    out, _ = _run(**inputs)
    return out

